# revision 18
# baseline (speedup 1.0000x reference)
"""MoE top-2 routing kernel for Trainium2 (8 NeuronCores, batch-sharded).

Problem (hardcoded shapes):
    x [8192, 3072] f32, Wg [3072, 8], bg [8], W1 [8, 3072, 128], b1 [8, 128],
    W2 [8, 128, 10], b2 [8, 10]  ->  out [8192, 10] f32
    g = x@Wg + bg; top-2 softmax over selected logits;
    y = sum_k w_k * (relu(x@W1[e_k] + b1[e_k]) @ W2[e_k] + b2[e_k])

Design (per core, 1024 tokens, dense over experts, PE-bound):
  - Host precasts x/W1/W2/Wg to fp16 (10-bit mantissa ~ tf32 quality).
    The DMA XBAR transposes x chunks straight into SBUF on the sync
    queue - no PE transposes, no PSUM staging, no fixup copies.  (The
    XBAR is a single shared unit: concurrent transpose streams from
    two engines corrupt each other, so everything stays on sync.)
  - Gating: stationary [Wg_hi | Wg_lo] (16 wide, host-built) removes
    the Wg rounding error; one matmul per chunk -> [16, 512] PSUM.
    The hi+lo fold happens token-major after the per-group transpose.
    fp16 x leaves a ~2^-11 logit error: top-2 selection flips exactly
    1 of 8192 tokens vs the fp32 reference (validated numerically);
    total rel err 7.3e-3 vs the 2e-2 gate.
  - Expert h matmuls in fp16 (1 cycle/row at N=512), fp32 PSUM.  The
    first 4 experts' accumulation interleaves with the front phase so
    the PE stays busy while the XBAR streams tile 0 in.
  - relu(+b1) runs on the DVE as (h + b1) max 0 - the scalar engine
    is dedicated to x_lo transposes.
  - W2 per expert is zero-padded into a [128, 80] stationary (expert e
    occupies columns 10e..10e+10): the 8 y matmuls accumulate disjoint
    10-partition stripes of ONE [80, 512] PSUM bank.  Per 128-token
    group: one PE transpose [80,128]->[128,80], then DVE: x w (free-dim
    broadcast over O), tree-sum over experts -> [128, 10].  b2 is a
    per-partition bias on the PSUM drain.
"""
import sys

for _p in ("/opt/trn_rl_repo",):
    if _p not in sys.path:
        sys.path.insert(0, _p)

import numpy as np
from contextlib import ExitStack

import concourse.bass as bass
import concourse.bacc as bacc
import concourse.tile as tile
import concourse.mybir as mybir
from concourse import bass_utils, masks

F32 = mybir.dt.float32
F16 = mybir.dt.float16
AF = mybir.ActivationFunctionType
OP = mybir.AluOpType

B, D, H, O, NE = 8192, 3072, 128, 10, 8
NCORES = 8
BC = B // NCORES          # tokens per core
TT = 512                  # token tile
NT = BC // TT             # token tiles per core
NCH = D // 128            # contraction chunks
NG = TT // 128            # 128-token groups per tile
NPRE = 4                  # experts whose h-accum interleaves the front

_CACHE = {}


def _build_program():
    nc = bacc.Bacc("TRN2", target_bir_lowering=False, debug=False,
                   num_devices=NCORES)

    x16 = nc.dram_tensor("x16", [BC, D], F16, kind="ExternalInput").ap()
    wgp = nc.dram_tensor("Wgp", [128, NCH, 2 * NE], F16, kind="ExternalInput").ap()
    bg = nc.dram_tensor("bg", [NE], F32, kind="ExternalInput").ap()
    w1 = nc.dram_tensor("W1b", [NE, 128, NCH, H], F16, kind="ExternalInput").ap()
    b1 = nc.dram_tensor("b1t", [H, NE], F32, kind="ExternalInput").ap()
    w2s = nc.dram_tensor("W2s", [H, NE, NE * O], F16, kind="ExternalInput").ap()
    b2c = nc.dram_tensor("b2c", [NE * O], F32, kind="ExternalInput").ap()
    out = nc.dram_tensor("out", [BC, O], F32, kind="ExternalOutput").ap()

    with tile.TileContext(nc) as tc:
        with ExitStack() as ctx:
            _kernel_body(ctx, tc, nc, x16, wgp, bg, w1, b1, w2s, b2c, out)
    nc.compile()
    return nc


def _kernel_body(ctx, tc, nc, x16, wgp, bg, w1, b1, w2s, b2c, out):
    singles = ctx.enter_context(tc.tile_pool(name="singles", bufs=1))
    xt_p = ctx.enter_context(tc.tile_pool(name="xt", bufs=2))
    gate_p = ctx.enter_context(tc.tile_pool(name="gate", bufs=2))
    hr_p = ctx.enter_context(tc.tile_pool(name="hr", bufs=2))
    yout_p = ctx.enter_context(tc.tile_pool(name="yout", bufs=2))

    ps_g = ctx.enter_context(tc.tile_pool(name="ps_g", bufs=1, space="PSUM"))
    ps_h = ctx.enter_context(tc.tile_pool(name="ps_h", bufs=NPRE, space="PSUM"))
    ps_y = ctx.enter_context(tc.tile_pool(name="ps_y", bufs=1, space="PSUM"))
    ps_s = ctx.enter_context(tc.tile_pool(name="ps_s", bufs=2, space="PSUM"))

    # ---- constants (small, on the gpsimd DGE queue) ----
    ident = singles.tile([128, 128], F32)
    masks.make_identity(nc, ident[:])

    wg_pair = singles.tile([128, NCH, 2 * NE], F16)
    nc.gpsimd.dma_start(wg_pair[:], wgp)
    # bg replicated across partitions (token-major bias for the epilogue)
    bg_row = singles.tile([1, NE], F32)
    nc.gpsimd.dma_start(bg_row[:], bg.rearrange("(one e) -> one e", one=1))
    bg_rep = singles.tile([128, NE], F32)
    nc.gpsimd.partition_broadcast(bg_rep[:], bg_row[:])
    b1t_sb = singles.tile([H, NE], F32)
    nc.gpsimd.dma_start(b1t_sb[:], b1)
    b2col = singles.tile([NE * O, 1], F32)
    nc.gpsimd.dma_start(b2col[:], b2c.rearrange("(p one) -> p one", one=1))
    w2st = singles.tile([H, NE, NE * O], F16)
    nc.gpsimd.dma_start(w2st[:], w2s)

    # ---- W1 resident fp16 (half the HBM traffic of fp32) ----
    # Loads ride the sync queue, explicitly ordered against the XBAR
    # transposes: W1[0..NPRE-1] first (the pre-experts need them from
    # chunk 0), W1[NPRE..] after tile-0's transposes (emitted in the
    # tile loop below) so they never stall the tile-0 fill.
    w1_sb = [
        singles.tile([128, NCH, H], F16, tag=f"w1_{e}", name=f"w1_{e}")
        for e in range(NE)
    ]

    def load_w1(e):
        nc.sync.dma_start(w1_sb[e][:], w1[e])

    load_w1(0)
    # W1[e] DMA issues interleave with tile-0's transposes on the sync
    # stream (transfers overlap the XBAR in the queues); pre-expert e's
    # h-accumulation starts lazily once its weights have landed
    W1_EMIT = {1: 1, 4: 2, 7: 3, 10: 4, 13: 5, 16: 6, 19: 7}  # chunk -> expert
    PRE_START = [0, 4, 7, 10]

    for t in range(NT):
        tok0 = t * TT

        # ---- front: XBAR-transpose x chunks, gate, pre-experts ----
        xts = []
        g_ps = ps_g.tile([2 * NE, TT], F32, tag="g")
        h_pre = [
            ps_h.tile([128, TT], F32, tag="h", name=f"hpre{t}_{e}")
            for e in range(NPRE)
        ]
        done = [-1] * NPRE
        for c in range(NCH):
            xt = xt_p.tile([128, TT], F16, tag=f"xt{c}", name=f"xt{c}")
            nc.sync.dma_start(
                xt[:], x16[tok0 : tok0 + TT, c * 128 : (c + 1) * 128],
                transpose=True,
            )
            if t == 0 and c in W1_EMIT:
                load_w1(W1_EMIT[c])
            xts.append(xt)
            nc.tensor.matmul(
                g_ps[:], wg_pair[:, c, :], xt[:],
                start=(c == 0), stop=(c == NCH - 1),
            )
            for e in range(NPRE):
                if t > 0 or c >= PRE_START[e]:
                    for cc in range(done[e] + 1, c + 1):
                        nc.tensor.matmul(
                            h_pre[e][:], w1_sb[e][:, cc, :], xts[cc][:],
                            start=(cc == 0), stop=(cc == NCH - 1),
                        )
                    done[e] = c

        # drain gating PSUM (partition-0 read; the hi/lo fold happens in
        # token-major after the per-group transpose)
        g_sb = gate_p.tile([2 * NE, TT], F32, tag="gsb")
        nc.vector.tensor_copy(g_sb[:], g_ps[:])

        # relus for the pre-experts go first on the DVE so expert 4's
        # PSUM bank frees up before the PE reaches it
        hrs = {}
        for e in range(NPRE):
            hr = hr_p.tile([128, TT], F16, tag="hr", name=f"hr{t}_{e}")
            nc.vector.tensor_scalar(
                hr[:], h_pre[e][:], b1t_sb[:, e : e + 1], 0.0, OP.add, OP.max
            )
            hrs[e] = hr

        y_ps = ps_y.tile([NE * O, TT], F32, tag="y")
        wfull = gate_p.tile([128, NG * NE], F32, tag="wfull")

        def w2_mm(e):
            nc.tensor.matmul(
                y_ps[:], w2st[:, e, :], hrs.pop(e)[:],
                start=(e == 0), stop=(e == NE - 1), skip_group_check=True,
            )

        def epilogue():
            # top-2 -> per-(token, expert) combine weights [128, NG*NE]
            for gg in range(NG):
                gt_ps = ps_s.tile([128, 2 * NE], F32, tag="s")
                nc.tensor.transpose(
                    gt_ps[:], g_sb[:, gg * 128 : (gg + 1) * 128],
                    ident[0 : 2 * NE, 0 : 2 * NE],
                )
                # fold hi+lo columns, add bg (one PSUM read per op)
                gth = gate_p.tile([128, NE], F32, tag="gth")
                nc.vector.tensor_add(gth[:], gt_ps[:, 0:NE], bg_rep[:])
                gt = gate_p.tile([128, NE], F32, tag="gt")
                nc.vector.tensor_add(gt[:], gt_ps[:, NE : 2 * NE], gth[:])

                maxs = gate_p.tile([128, 8], F32, tag="maxs")
                nc.vector.max(maxs[:], gt[:])
                top1, top2 = maxs[:, 0:1], maxs[:, 1:2]

                sm = gate_p.tile([128, 4], F32, tag="sm")
                d21, e21, den, w2c = (sm[:, i : i + 1] for i in range(4))
                nc.vector.tensor_sub(d21, top2, top1)
                nc.scalar.activation(e21, d21, AF.Exp)
                nc.vector.tensor_scalar(den, e21, 1.0, None, OP.add)
                w1c = gate_p.tile([128, 1], F32, tag="w1c")
                nc.vector.reciprocal(w1c[:], den)
                nc.vector.tensor_mul(w2c, e21, w1c[:])

                m1 = gate_p.tile([128, NE], F32, tag="m1")
                m2 = gate_p.tile([128, NE], F32, tag="m2")
                nc.vector.tensor_scalar(m1[:], gt[:], top1, None, OP.is_equal)
                nc.vector.tensor_scalar(m2[:], gt[:], top2, None, OP.is_equal)
                nc.vector.tensor_scalar(m1[:], m1[:], w1c[:, 0:1], None, OP.mult)
                nc.vector.tensor_scalar(m2[:], m2[:], w2c, None, OP.mult)
                nc.vector.tensor_add(
                    wfull[:, gg * NE : (gg + 1) * NE], m1[:], m2[:]
                )

        # ---- experts 4..7 + deferred W2 matmuls + epilogue ----
        # PE order: h4, gtT(epilogue), h5, W2[0..2], h6, W2[3..4],
        #           h7, W2[5], W2[6], W2[7]
        for e in range(NPRE, NE):
            h_ps = ps_h.tile([128, TT], F32, tag="h")
            for c in range(NCH):
                nc.tensor.matmul(
                    h_ps[:], w1_sb[e][:, c, :], xts[c][:],
                    start=(c == 0), stop=(c == NCH - 1),
                )
            if e == NPRE:
                epilogue()
            elif e == NPRE + 1:
                for ee in range(3):
                    w2_mm(ee)
            elif e == NPRE + 2:
                w2_mm(3)
                w2_mm(4)
            else:
                w2_mm(5)
            hr = hr_p.tile([128, TT], F16, tag="hr", name=f"hr{t}_{e}")
            nc.vector.tensor_scalar(
                hr[:], h_ps[:], b1t_sb[:, e : e + 1], 0.0, OP.add, OP.max
            )
            hrs[e] = hr
        w2_mm(6)
        w2_mm(7)

        # ---- combine: drain (+b2), transpose, x w, tree-sum ----
        y_sb = yout_p.tile([NE * O, TT], F32, tag="ysb")
        nc.vector.tensor_scalar(y_sb[:], y_ps[:], b2col[:, 0:1], None, OP.add)

        yt_acc = yout_p.tile([128, NG * O], F32, tag="ytacc")
        for gg in range(NG):
            yt_ps = ps_s.tile([128, NE * O], F32, tag="s")
            nc.tensor.transpose(
                yt_ps[:], y_sb[:, gg * 128 : (gg + 1) * 128],
                ident[0 : NE * O, 0 : NE * O],
            )
            sc = yout_p.tile([128, NE, O], F32, tag="sc")
            w_bc = (
                wfull[:, gg * NE : (gg + 1) * NE]
                .unsqueeze(2)
                .broadcast_to([128, NE, O])
            )
            nc.vector.tensor_tensor(
                sc[:], yt_ps[:].rearrange("p (e o) -> p e o", e=NE), w_bc,
                op=OP.mult,
            )
            f1 = yout_p.tile([128, 4 * O], F32, tag="f1")
            nc.vector.tensor_add(f1[:], sc[:, 0:4, :], sc[:, 4:8, :])
            f2 = yout_p.tile([128, 2 * O], F32, tag="f2")
            nc.vector.tensor_add(
                f2[:], f1[:, 0 : 2 * O], f1[:, 2 * O : 4 * O]
            )
            nc.vector.tensor_add(
                yt_acc[:, gg * O : (gg + 1) * O],
                f2[:, 0:O], f2[:, O : 2 * O],
            )

        nc.gpsimd.dma_start(
            out[tok0 : tok0 + TT].rearrange("(gg p) o -> p gg o", p=128),
            yt_acc[:].rearrange("p (gg o) -> p gg o", gg=NG),
        )


def _get_nc():
    if "nc" not in _CACHE:
        _CACHE["nc"] = _build_program()
    return _CACHE["nc"]


def _f16(a):
    return np.asarray(a, dtype=np.float32).astype(np.float16)


def kernel(x, Wg, bg, W1, b1, W2, b2, _trace=False, _tmpdir=None):
    nc = _get_nc()
    x = np.ascontiguousarray(np.asarray(x, dtype=np.float32))
    x_16 = _f16(x)

    Wg = np.asarray(Wg, dtype=np.float32)
    wg_hi = _f16(Wg)
    wg_lo = _f16(Wg - wg_hi.astype(np.float32))
    wgp = np.concatenate([wg_hi, wg_lo], axis=1)          # [D, 16]
    wgp = wgp.reshape(NCH, 128, 2 * NE).transpose(1, 0, 2)  # [128, NCH, 16]

    W2 = np.asarray(W2, dtype=np.float32)
    w2s = np.zeros((H, NE, NE * O), dtype=np.float16)
    for e in range(NE):
        w2s[:, e, O * e : O * (e + 1)] = _f16(W2[e])

    w1b = _f16(W1).reshape(NE, NCH, 128, H).transpose(0, 2, 1, 3)

    shared = {
        "Wgp": np.ascontiguousarray(wgp),
        "bg": np.ascontiguousarray(np.asarray(bg, dtype=np.float32)),
        "W1b": np.ascontiguousarray(w1b),
        "b1t": np.ascontiguousarray(np.asarray(b1, dtype=np.float32).T),
        "W2s": np.ascontiguousarray(w2s),
        "b2c": np.ascontiguousarray(
            np.asarray(b2, dtype=np.float32).reshape(NE * O)
        ),
    }
    in_maps = [
        {"x16": x_16[c * BC : (c + 1) * BC], **shared} for c in range(NCORES)
    ]
    res = bass_utils.run_bass_kernel_spmd(
        nc,
        in_maps,
        core_ids=list(range(NCORES)),
        trace=_trace,
        tmpdir=_tmpdir,
    )
    outp = np.concatenate([res.results[c]["out"] for c in range(NCORES)], axis=0)
    if _trace:
        kernel._last_results = res
    return outp


# revision 21
# speedup vs baseline: 1.1841x; 1.1841x over previous
"""MoE top-2 routing kernel for Trainium2 (8 NeuronCores, batch-sharded).

Problem (hardcoded shapes):
    x [8192, 3072] f32, Wg [3072, 8], bg [8], W1 [8, 3072, 128], b1 [8, 128],
    W2 [8, 128, 10], b2 [8, 10]  ->  out [8192, 10] f32
    g = x@Wg + bg; top-2 softmax over selected logits;
    y = sum_k w_k * (relu(x@W1[e_k] + b1[e_k]) @ W2[e_k] + b2[e_k])

Design (per core, 1024 tokens, dense over experts, PE-bound):
  - Host precasts x/W1/W2/Wg to fp16 (10-bit mantissa ~ tf32 quality)
    and pre-arranges every weight layout so all DMAs are contiguous.
  - x chunks are PE-transposed (fp16 moving, 1 cycle/row) and drained
    to SBUF fp16 by the scalar engine (values are already fp16 -
    the drain is exact).
  - Gating: stationary [Wg_hi | Wg_lo] (16 wide, host-built) removes
    the Wg rounding error; ONE matmul per chunk -> [16, 512] PSUM.
    The hi+lo fold happens token-major after the per-group transpose
    (DVE reads must start at partition 0).  fp16 x leaves a ~2^-11
    logit error: top-2 selection flips exactly 1 of 8192 tokens vs
    the fp32 reference (validated numerically); rel err 7.3e-3 vs
    the 2e-2 gate.
  - Expert h matmuls in fp16, fp32 PSUM.  relu(+b1) runs on the DVE
    as (h + b1) max 0.
  - W2 per expert is zero-padded into a [128, 80] stationary (expert e
    occupies columns 10e..10e+10): the 8 y matmuls accumulate disjoint
    10-partition stripes of ONE [80, 512] PSUM bank.  Per 128-token
    group: one PE transpose [80,128]->[128,80], then DVE: x w (free-dim
    broadcast over O), tree-sum over experts -> [128, 10].  b2 is a
    per-partition bias on the PSUM drain.
  - Software pipeline: tile t+1's front phase (x loads, transposes,
    gating) is interleaved into tile t's expert phase, so the PE never
    waits for data after the first tile fills.  W1 loads ride the sync
    queue between tile-0's x and tile-1's x, in expert order, so the
    expert loop streams behind the arrivals.
"""
import sys

for _p in ("/opt/trn_rl_repo",):
    if _p not in sys.path:
        sys.path.insert(0, _p)

import numpy as np
from contextlib import ExitStack

import concourse.bass as bass
import concourse.bacc as bacc
import concourse.tile as tile
import concourse.mybir as mybir
from concourse import bass_utils, masks

F32 = mybir.dt.float32
F16 = mybir.dt.float16
AF = mybir.ActivationFunctionType
OP = mybir.AluOpType

B, D, H, O, NE = 8192, 3072, 128, 10, 8
NCORES = 8
BC = B // NCORES          # tokens per core
TT = 512                  # token tile
NT = BC // TT             # token tiles per core
NCH = D // 128            # contraction chunks
NG = TT // 128            # 128-token groups per tile
SKEW = 3                  # gating trails the transposes by SKEW chunks

_CACHE = {}


def _build_program():
    nc = bacc.Bacc("TRN2", target_bir_lowering=False, debug=False,
                   num_devices=NCORES)

    x16 = nc.dram_tensor("x16", [BC, D], F16, kind="ExternalInput").ap()
    wgp = nc.dram_tensor("Wgp", [128, NCH, 2 * NE], F16, kind="ExternalInput").ap()
    bg = nc.dram_tensor("bg", [NE], F32, kind="ExternalInput").ap()
    w1 = nc.dram_tensor("W1b", [NE, 128, NCH, H], F16, kind="ExternalInput").ap()
    b1 = nc.dram_tensor("b1t", [H, NE], F32, kind="ExternalInput").ap()
    w2s = nc.dram_tensor("W2s", [H, NE, NE * O], F16, kind="ExternalInput").ap()
    b2c = nc.dram_tensor("b2c", [NE * O], F32, kind="ExternalInput").ap()
    out = nc.dram_tensor("out", [BC, O], F32, kind="ExternalOutput").ap()

    with tile.TileContext(nc) as tc:
        with ExitStack() as ctx:
            _kernel_body(ctx, tc, nc, x16, wgp, bg, w1, b1, w2s, b2c, out)
    nc.compile()
    return nc


def _kernel_body(ctx, tc, nc, x16, wgp, bg, w1, b1, w2s, b2c, out):
    singles = ctx.enter_context(tc.tile_pool(name="singles", bufs=1))
    xin_p = ctx.enter_context(tc.tile_pool(name="xin", bufs=4))
    xt_p = ctx.enter_context(tc.tile_pool(name="xt", bufs=2))
    gate_p = ctx.enter_context(tc.tile_pool(name="gate", bufs=2))
    hr_p = ctx.enter_context(tc.tile_pool(name="hr", bufs=2))
    yout_p = ctx.enter_context(tc.tile_pool(name="yout", bufs=2))

    ps_xtp = ctx.enter_context(tc.tile_pool(name="ps_xtp", bufs=2, space="PSUM"))
    ps_g = ctx.enter_context(tc.tile_pool(name="ps_g", bufs=1, space="PSUM"))
    ps_h = ctx.enter_context(tc.tile_pool(name="ps_h", bufs=2, space="PSUM"))
    ps_y = ctx.enter_context(tc.tile_pool(name="ps_y", bufs=1, space="PSUM"))
    ps_s = ctx.enter_context(tc.tile_pool(name="ps_s", bufs=2, space="PSUM"))

    # ---- constants (small, on the gpsimd DGE queue) ----
    ident = singles.tile([128, 128], F32)
    masks.make_identity(nc, ident[:])
    ident16 = singles.tile([128, 128], F16)
    nc.vector.tensor_copy(ident16[:], ident[:])

    wg_pair = singles.tile([128, NCH, 2 * NE], F16)
    nc.gpsimd.dma_start(wg_pair[:], wgp)
    bg_row = singles.tile([1, NE], F32)
    nc.gpsimd.dma_start(bg_row[:], bg.rearrange("(one e) -> one e", one=1))
    bg_rep = singles.tile([128, NE], F32)
    nc.gpsimd.partition_broadcast(bg_rep[:], bg_row[:])
    b1t_sb = singles.tile([H, NE], F32)
    nc.gpsimd.dma_start(b1t_sb[:], b1)
    b2col = singles.tile([NE * O, 1], F32)
    nc.gpsimd.dma_start(b2col[:], b2c.rearrange("(p one) -> p one", one=1))
    w2st = singles.tile([H, NE, NE * O], F16)
    nc.gpsimd.dma_start(w2st[:], w2s)

    w1_sb = [
        singles.tile([128, NCH, H], F16, tag=f"w1_{e}", name=f"w1_{e}")
        for e in range(NE)
    ]

    def load_w1(e):
        nc.sync.dma_start(w1_sb[e][:], w1[e])

    # ---------------- pipeline stage generators ----------------
    # front(t): per chunk c yields after emitting [xin dma, 4 PE
    # transposes, scalar drain, gating(c-SKEW)]
    def front(t):
        tok0 = t * TT
        xts = state[t]["xts"]
        g_ps = ps_g.tile([2 * NE, TT], F32, tag="g", name=f"g{t}")
        state[t]["g_ps"] = g_ps

        def gating(cg):
            nc.tensor.matmul(
                g_ps[:], wg_pair[:, cg, :], xts[cg][:],
                start=(cg == 0), stop=(cg == NCH - 1),
            )

        for c in range(NCH):
            xin = xin_p.tile([128, NG, 128], F16, tag="xin")
            nc.sync.dma_start(
                xin[:],
                x16[tok0 : tok0 + TT, c * 128 : (c + 1) * 128].rearrange(
                    "(gg p) d -> p gg d", p=128
                ),
            )
            if t == 0 and c == 18:
                load_w1(0)  # W1[0] rides near the end of tile-0's x
            xtp = ps_xtp.tile([128, TT], F16, tag="xtp")
            for gg in range(NG):
                nc.tensor.matmul(
                    xtp[:, gg * 128 : (gg + 1) * 128],
                    xin[:, gg, :],
                    ident16[:],
                    is_transpose=True,
                    start=True,
                    stop=True,
                    skip_group_check=True,
                )
            xt = xt_p.tile([128, TT], F16, tag=f"xt{c}", name=f"xt{c}")
            nc.scalar.copy(xt[:], xtp[:])
            xts.append(xt)
            if c >= SKEW:
                gating(c - SKEW)
            yield
        for cg in range(NCH - SKEW, NCH):
            gating(cg)

    # experts(t): yields between experts; interleaves the epilogue and
    # the deferred W2 matmuls exactly as the PE should see them
    def experts(t):
        xts = state[t]["xts"]
        g_ps = state[t]["g_ps"]

        g_sb = gate_p.tile([2 * NE, TT], F32, tag="gsb")
        nc.vector.tensor_copy(g_sb[:], g_ps[:])

        y_ps = ps_y.tile([NE * O, TT], F32, tag="y", name=f"y{t}")
        wfull = gate_p.tile([128, NG * NE], F32, tag="wfull")
        state[t]["y_ps"] = y_ps
        state[t]["wfull"] = wfull
        hrs = {}

        def w2_mm(e):
            nc.tensor.matmul(
                y_ps[:], w2st[:, e, :], hrs.pop(e)[:],
                start=(e == 0), stop=(e == NE - 1), skip_group_check=True,
            )

        def epilogue():
            for gg in range(NG):
                gt_ps = ps_s.tile([128, 2 * NE], F32, tag="s")
                nc.tensor.transpose(
                    gt_ps[:], g_sb[:, gg * 128 : (gg + 1) * 128],
                    ident[0 : 2 * NE, 0 : 2 * NE],
                )
                gth = gate_p.tile([128, NE], F32, tag="gth")
                nc.vector.tensor_add(gth[:], gt_ps[:, 0:NE], bg_rep[:])
                gt = gate_p.tile([128, NE], F32, tag="gt")
                nc.vector.tensor_add(gt[:], gt_ps[:, NE : 2 * NE], gth[:])

                maxs = gate_p.tile([128, 8], F32, tag="maxs")
                nc.vector.max(maxs[:], gt[:])
                top1, top2 = maxs[:, 0:1], maxs[:, 1:2]

                sm = gate_p.tile([128, 4], F32, tag="sm")
                d21, e21, den, w2c = (sm[:, i : i + 1] for i in range(4))
                nc.vector.tensor_sub(d21, top2, top1)
                nc.scalar.activation(e21, d21, AF.Exp)
                nc.vector.tensor_scalar(den, e21, 1.0, None, OP.add)
                w1c = gate_p.tile([128, 1], F32, tag="w1c")
                nc.vector.reciprocal(w1c[:], den)
                nc.vector.tensor_mul(w2c, e21, w1c[:])

                m1 = gate_p.tile([128, NE], F32, tag="m1")
                m2 = gate_p.tile([128, NE], F32, tag="m2")
                nc.vector.tensor_scalar(m1[:], gt[:], top1, None, OP.is_equal)
                nc.vector.tensor_scalar(m2[:], gt[:], top2, None, OP.is_equal)
                nc.vector.tensor_scalar(m1[:], m1[:], w1c[:, 0:1], None, OP.mult)
                nc.vector.tensor_scalar(m2[:], m2[:], w2c, None, OP.mult)
                nc.vector.tensor_add(
                    wfull[:, gg * NE : (gg + 1) * NE], m1[:], m2[:]
                )

        for e in range(NE):
            h_ps = ps_h.tile([128, TT], F32, tag="h")
            for c in range(NCH):
                nc.tensor.matmul(
                    h_ps[:], w1_sb[e][:, c, :], xts[c][:],
                    start=(c == 0), stop=(c == NCH - 1),
                )
            if e == 0:
                epilogue()
            elif e >= 2:
                w2_mm(e - 2)
            hr = hr_p.tile([128, TT], F16, tag="hr", name=f"hr{t}_{e}")
            nc.vector.tensor_scalar(
                hr[:], h_ps[:], b1t_sb[:, e : e + 1], 0.0, OP.add, OP.max
            )
            hrs[e] = hr
            yield
        w2_mm(NE - 2)
        w2_mm(NE - 1)

    def combine(t):
        wfull = state[t]["wfull"]
        y_sb = yout_p.tile([NE * O, TT], F32, tag="ysb")
        nc.vector.tensor_scalar(
            y_sb[:], state[t]["y_ps"][:], b2col[:, 0:1], None, OP.add
        )
        yt_acc = yout_p.tile([128, NG * O], F32, tag="ytacc")
        for gg in range(NG):
            yt_ps = ps_s.tile([128, NE * O], F32, tag="s")
            nc.tensor.transpose(
                yt_ps[:], y_sb[:, gg * 128 : (gg + 1) * 128],
                ident[0 : NE * O, 0 : NE * O],
            )
            sc = yout_p.tile([128, NE, O], F32, tag="sc")
            w_bc = (
                wfull[:, gg * NE : (gg + 1) * NE]
                .unsqueeze(2)
                .broadcast_to([128, NE, O])
            )
            nc.vector.tensor_tensor(
                sc[:], yt_ps[:].rearrange("p (e o) -> p e o", e=NE), w_bc,
                op=OP.mult,
            )
            f1 = yout_p.tile([128, 4 * O], F32, tag="f1")
            nc.vector.tensor_add(f1[:], sc[:, 0:4, :], sc[:, 4:8, :])
            f2 = yout_p.tile([128, 2 * O], F32, tag="f2")
            nc.vector.tensor_add(
                f2[:], f1[:, 0 : 2 * O], f1[:, 2 * O : 4 * O]
            )
            nc.vector.tensor_add(
                yt_acc[:, gg * O : (gg + 1) * O],
                f2[:, 0:O], f2[:, O : 2 * O],
            )
        nc.gpsimd.dma_start(
            out[t * TT : (t + 1) * TT].rearrange("(gg p) o -> p gg o", p=128),
            yt_acc[:].rearrange("p (gg o) -> p gg o", gg=NG),
        )

    # ---------------- pipeline driver ----------------
    state = [dict(xts=[]) for _ in range(NT)]

    # tile 0 front runs alone (nothing to overlap with)
    for _ in front(0):
        pass
    # W1[1..7] DMAs enqueue between tile-0's and tile-1's x loads
    for e in range(1, NE):
        load_w1(e)

    for t in range(NT):
        exp_gen = experts(t)
        if t + 1 < NT:
            # interleave the next tile's front (24 chunk-steps) into this
            # tile's expert phase (8 expert-steps): 3 chunks per expert
            fr_gen = front(t + 1)
            for ei in range(NE):
                next(exp_gen, None)
                for _ in range(3):
                    next(fr_gen, None)
            for _ in fr_gen:
                pass
        else:
            for _ in exp_gen:
                pass
        # drain the generator (emits the trailing W2 matmuls)
        for _ in exp_gen:
            pass
        combine(t)


def _get_nc():
    if "nc" not in _CACHE:
        _CACHE["nc"] = _build_program()
    return _CACHE["nc"]


def _f16(a):
    return np.asarray(a, dtype=np.float32).astype(np.float16)


def kernel(x, Wg, bg, W1, b1, W2, b2, _trace=False, _tmpdir=None):
    nc = _get_nc()
    x = np.ascontiguousarray(np.asarray(x, dtype=np.float32))
    x_16 = _f16(x)

    Wg = np.asarray(Wg, dtype=np.float32)
    wg_hi = _f16(Wg)
    wg_lo = _f16(Wg - wg_hi.astype(np.float32))
    wgp = np.concatenate([wg_hi, wg_lo], axis=1)          # [D, 16]
    wgp = wgp.reshape(NCH, 128, 2 * NE).transpose(1, 0, 2)  # [128, NCH, 16]

    W2 = np.asarray(W2, dtype=np.float32)
    w2s = np.zeros((H, NE, NE * O), dtype=np.float16)
    for e in range(NE):
        w2s[:, e, O * e : O * (e + 1)] = _f16(W2[e])

    w1b = _f16(W1).reshape(NE, NCH, 128, H).transpose(0, 2, 1, 3)

    shared = {
        "Wgp": np.ascontiguousarray(wgp),
        "bg": np.ascontiguousarray(np.asarray(bg, dtype=np.float32)),
        "W1b": np.ascontiguousarray(w1b),
        "b1t": np.ascontiguousarray(np.asarray(b1, dtype=np.float32).T),
        "W2s": np.ascontiguousarray(w2s),
        "b2c": np.ascontiguousarray(
            np.asarray(b2, dtype=np.float32).reshape(NE * O)
        ),
    }
    in_maps = [
        {"x16": x_16[c * BC : (c + 1) * BC], **shared} for c in range(NCORES)
    ]
    res = bass_utils.run_bass_kernel_spmd(
        nc,
        in_maps,
        core_ids=list(range(NCORES)),
        trace=_trace,
        tmpdir=_tmpdir,
    )
    outp = np.concatenate([res.results[c]["out"] for c in range(NCORES)], axis=0)
    if _trace:
        kernel._last_results = res
    return outp


# revision 22
# speedup vs baseline: 1.2022x; 1.0153x over previous
"""MoE top-2 routing kernel for Trainium2 (8 NeuronCores, batch-sharded).

Problem (hardcoded shapes):
    x [8192, 3072] f32, Wg [3072, 8], bg [8], W1 [8, 3072, 128], b1 [8, 128],
    W2 [8, 128, 10], b2 [8, 10]  ->  out [8192, 10] f32
    g = x@Wg + bg; top-2 softmax over selected logits;
    y = sum_k w_k * (relu(x@W1[e_k] + b1[e_k]) @ W2[e_k] + b2[e_k])

Design (per core, 1024 tokens, dense over experts, PE-bound):
  - Host precasts x/W1/W2/Wg to fp16 (10-bit mantissa ~ tf32 quality)
    and pre-arranges every weight layout so all DMAs are contiguous.
  - x chunks are PE-transposed (fp16 moving, 1 cycle/row) and drained
    to SBUF fp16 by the scalar engine (values are already fp16 -
    the drain is exact).
  - Gating: stationary [Wg_hi | Wg_lo] (16 wide, host-built) removes
    the Wg rounding error; ONE matmul per chunk -> [16, 512] PSUM.
    The hi+lo fold happens token-major after the per-group transpose
    (DVE reads must start at partition 0).  fp16 x leaves a ~2^-11
    logit error: top-2 selection flips exactly 1 of 8192 tokens vs
    the fp32 reference (validated numerically); rel err 7.3e-3 vs
    the 2e-2 gate.
  - Expert h matmuls in fp16, fp32 PSUM.  relu(+b1) runs on the DVE
    as (h + b1) max 0.
  - W2 per expert is zero-padded into a [128, 80] stationary (expert e
    occupies columns 10e..10e+10): the 8 y matmuls accumulate disjoint
    10-partition stripes of ONE [80, 512] PSUM bank.  Per 128-token
    group: one PE transpose [80,128]->[128,80], then DVE: x w (free-dim
    broadcast over O), tree-sum over experts -> [128, 10].  b2 is a
    per-partition bias on the PSUM drain.
  - Software pipeline: tile t+1's front phase (x loads, transposes,
    gating) is interleaved into tile t's expert phase, so the PE never
    waits for data after the first tile fills.  W1 loads ride the sync
    queue between tile-0's x and tile-1's x, in expert order, so the
    expert loop streams behind the arrivals.
"""
import sys

for _p in ("/opt/trn_rl_repo",):
    if _p not in sys.path:
        sys.path.insert(0, _p)

import numpy as np
from contextlib import ExitStack

import concourse.bass as bass
import concourse.bacc as bacc
import concourse.tile as tile
import concourse.mybir as mybir
from concourse import bass_utils, masks

F32 = mybir.dt.float32
F16 = mybir.dt.float16
AF = mybir.ActivationFunctionType
OP = mybir.AluOpType

B, D, H, O, NE = 8192, 3072, 128, 10, 8
NCORES = 8
BC = B // NCORES          # tokens per core
TT = 512                  # token tile
NT = BC // TT             # token tiles per core
NCH = D // 128            # contraction chunks
NG = TT // 128            # 128-token groups per tile
SKEW = 3                  # gating trails the transposes by SKEW chunks

_CACHE = {}


def _build_program():
    nc = bacc.Bacc("TRN2", target_bir_lowering=False, debug=False,
                   num_devices=NCORES)

    x16 = nc.dram_tensor("x16", [BC, D], F16, kind="ExternalInput").ap()
    wgp = nc.dram_tensor("Wgp", [128, NCH, 2 * NE], F16, kind="ExternalInput").ap()
    bg = nc.dram_tensor("bg", [NE], F32, kind="ExternalInput").ap()
    w1 = nc.dram_tensor("W1b", [NE, 128, NCH, H], F16, kind="ExternalInput").ap()
    b1 = nc.dram_tensor("b1t", [H, NE], F32, kind="ExternalInput").ap()
    w2s = nc.dram_tensor("W2s", [H, NE, NE * O], F16, kind="ExternalInput").ap()
    b2c = nc.dram_tensor("b2c", [NE * O], F32, kind="ExternalInput").ap()
    out = nc.dram_tensor("out", [BC, O], F32, kind="ExternalOutput").ap()

    with tile.TileContext(nc) as tc:
        with ExitStack() as ctx:
            _kernel_body(ctx, tc, nc, x16, wgp, bg, w1, b1, w2s, b2c, out)
    nc.compile()
    return nc


def _kernel_body(ctx, tc, nc, x16, wgp, bg, w1, b1, w2s, b2c, out):
    singles = ctx.enter_context(tc.tile_pool(name="singles", bufs=1))
    xin_p = ctx.enter_context(tc.tile_pool(name="xin", bufs=4))
    xt_p = ctx.enter_context(tc.tile_pool(name="xt", bufs=2))
    gate_p = ctx.enter_context(tc.tile_pool(name="gate", bufs=2))
    hr_p = ctx.enter_context(tc.tile_pool(name="hr", bufs=2))
    yout_p = ctx.enter_context(tc.tile_pool(name="yout", bufs=2))

    ps_xtp = ctx.enter_context(tc.tile_pool(name="ps_xtp", bufs=2, space="PSUM"))
    ps_g = ctx.enter_context(tc.tile_pool(name="ps_g", bufs=1, space="PSUM"))
    ps_h = ctx.enter_context(tc.tile_pool(name="ps_h", bufs=2, space="PSUM"))
    ps_y = ctx.enter_context(tc.tile_pool(name="ps_y", bufs=1, space="PSUM"))
    ps_s = ctx.enter_context(tc.tile_pool(name="ps_s", bufs=2, space="PSUM"))

    # ---- constants (small, on the gpsimd DGE queue) ----
    ident = singles.tile([128, 128], F32)
    masks.make_identity(nc, ident[:])
    ident16 = singles.tile([128, 128], F16)
    nc.vector.tensor_copy(ident16[:], ident[:])

    wg_pair = singles.tile([128, NCH, 2 * NE], F16)
    nc.gpsimd.dma_start(wg_pair[:], wgp)
    bg_row = singles.tile([1, NE], F32)
    nc.gpsimd.dma_start(bg_row[:], bg.rearrange("(one e) -> one e", one=1))
    bg_rep = singles.tile([128, NE], F32)
    nc.gpsimd.partition_broadcast(bg_rep[:], bg_row[:])
    b1t_sb = singles.tile([H, NE], F32)
    nc.gpsimd.dma_start(b1t_sb[:], b1)
    b2col = singles.tile([NE * O, 1], F32)
    nc.gpsimd.dma_start(b2col[:], b2c.rearrange("(p one) -> p one", one=1))
    w2st = singles.tile([H, NE, NE * O], F16)
    nc.gpsimd.dma_start(w2st[:], w2s)

    w1_sb = [
        singles.tile([128, NCH, H], F16, tag=f"w1_{e}", name=f"w1_{e}")
        for e in range(NE)
    ]

    def load_w1(e):
        nc.sync.dma_start(w1_sb[e][:], w1[e])

    # ---------------- pipeline stage generators ----------------
    # front(t): per chunk c yields after emitting the chunk's work.
    # Tile 0 loads token-major and PE-transposes (the XBAR can't beat
    # the PE into the pipeline at t=0); later tiles use the DMA XBAR
    # transpose on the sync queue - it runs during the previous tile's
    # expert phase, when the sync engine and queues are otherwise idle.
    # (The XBAR is a single shared unit - only ever one stream.)
    def front(t):
        tok0 = t * TT
        xts = state[t]["xts"]
        g_ps = ps_g.tile([2 * NE, TT], F32, tag="g", name=f"g{t}")
        state[t]["g_ps"] = g_ps

        def gating(cg):
            nc.tensor.matmul(
                g_ps[:], wg_pair[:, cg, :], xts[cg][:],
                start=(cg == 0), stop=(cg == NCH - 1),
            )

        if t > 0:
            for c in range(NCH):
                xt = xt_p.tile([128, TT], F16, tag=f"xt{c}", name=f"xt{c}")
                nc.sync.dma_start(
                    xt[:], x16[tok0 : tok0 + TT, c * 128 : (c + 1) * 128],
                    transpose=True,
                )
                xts.append(xt)
                gating(c)
                yield
            return

        for c in range(NCH):
            xin = xin_p.tile([128, NG, 128], F16, tag="xin")
            nc.sync.dma_start(
                xin[:],
                x16[tok0 : tok0 + TT, c * 128 : (c + 1) * 128].rearrange(
                    "(gg p) d -> p gg d", p=128
                ),
            )
            if c == 18:
                load_w1(0)  # W1[0] rides near the end of tile-0's x
            xtp = ps_xtp.tile([128, TT], F16, tag="xtp")
            for gg in range(NG):
                nc.tensor.matmul(
                    xtp[:, gg * 128 : (gg + 1) * 128],
                    xin[:, gg, :],
                    ident16[:],
                    is_transpose=True,
                    start=True,
                    stop=True,
                    skip_group_check=True,
                )
            xt = xt_p.tile([128, TT], F16, tag=f"xt{c}", name=f"xt{c}")
            nc.scalar.copy(xt[:], xtp[:])
            xts.append(xt)
            if c >= SKEW:
                gating(c - SKEW)
            yield
        for cg in range(NCH - SKEW, NCH):
            gating(cg)

    # experts(t): yields between experts; interleaves the epilogue and
    # the deferred W2 matmuls exactly as the PE should see them
    def experts(t):
        xts = state[t]["xts"]
        g_ps = state[t]["g_ps"]

        g_sb = gate_p.tile([2 * NE, TT], F32, tag="gsb")
        nc.vector.tensor_copy(g_sb[:], g_ps[:])

        y_ps = ps_y.tile([NE * O, TT], F32, tag="y", name=f"y{t}")
        wfull = gate_p.tile([128, NG * NE], F32, tag="wfull")
        state[t]["y_ps"] = y_ps
        state[t]["wfull"] = wfull
        hrs = {}

        def w2_mm(e):
            nc.tensor.matmul(
                y_ps[:], w2st[:, e, :], hrs.pop(e)[:],
                start=(e == 0), stop=(e == NE - 1), skip_group_check=True,
            )

        def epilogue():
            for gg in range(NG):
                gt_ps = ps_s.tile([128, 2 * NE], F32, tag="s")
                nc.tensor.transpose(
                    gt_ps[:], g_sb[:, gg * 128 : (gg + 1) * 128],
                    ident[0 : 2 * NE, 0 : 2 * NE],
                )
                gth = gate_p.tile([128, NE], F32, tag="gth")
                nc.vector.tensor_add(gth[:], gt_ps[:, 0:NE], bg_rep[:])
                gt = gate_p.tile([128, NE], F32, tag="gt")
                nc.vector.tensor_add(gt[:], gt_ps[:, NE : 2 * NE], gth[:])

                maxs = gate_p.tile([128, 8], F32, tag="maxs")
                nc.vector.max(maxs[:], gt[:])
                top1, top2 = maxs[:, 0:1], maxs[:, 1:2]

                sm = gate_p.tile([128, 4], F32, tag="sm")
                d21, e21, den, w2c = (sm[:, i : i + 1] for i in range(4))
                nc.vector.tensor_sub(d21, top2, top1)
                nc.scalar.activation(e21, d21, AF.Exp)
                nc.vector.tensor_scalar(den, e21, 1.0, None, OP.add)
                w1c = gate_p.tile([128, 1], F32, tag="w1c")
                nc.vector.reciprocal(w1c[:], den)
                nc.vector.tensor_mul(w2c, e21, w1c[:])

                m1 = gate_p.tile([128, NE], F32, tag="m1")
                m2 = gate_p.tile([128, NE], F32, tag="m2")
                nc.vector.tensor_scalar(m1[:], gt[:], top1, None, OP.is_equal)
                nc.vector.tensor_scalar(m2[:], gt[:], top2, None, OP.is_equal)
                nc.vector.tensor_scalar(m1[:], m1[:], w1c[:, 0:1], None, OP.mult)
                nc.vector.tensor_scalar(m2[:], m2[:], w2c, None, OP.mult)
                nc.vector.tensor_add(
                    wfull[:, gg * NE : (gg + 1) * NE], m1[:], m2[:]
                )

        for e in range(NE):
            h_ps = ps_h.tile([128, TT], F32, tag="h")
            for c in range(NCH):
                nc.tensor.matmul(
                    h_ps[:], w1_sb[e][:, c, :], xts[c][:],
                    start=(c == 0), stop=(c == NCH - 1),
                )
            if e == 0:
                epilogue()
            elif e >= 2:
                w2_mm(e - 2)
            hr = hr_p.tile([128, TT], F16, tag="hr", name=f"hr{t}_{e}")
            nc.vector.tensor_scalar(
                hr[:], h_ps[:], b1t_sb[:, e : e + 1], 0.0, OP.add, OP.max
            )
            hrs[e] = hr
            yield
        w2_mm(NE - 2)
        w2_mm(NE - 1)

    def combine(t):
        wfull = state[t]["wfull"]
        y_sb = yout_p.tile([NE * O, TT], F32, tag="ysb")
        nc.vector.tensor_scalar(
            y_sb[:], state[t]["y_ps"][:], b2col[:, 0:1], None, OP.add
        )
        yt_acc = yout_p.tile([128, NG * O], F32, tag="ytacc")
        for gg in range(NG):
            yt_ps = ps_s.tile([128, NE * O], F32, tag="s")
            nc.tensor.transpose(
                yt_ps[:], y_sb[:, gg * 128 : (gg + 1) * 128],
                ident[0 : NE * O, 0 : NE * O],
            )
            sc = yout_p.tile([128, NE, O], F32, tag="sc")
            w_bc = (
                wfull[:, gg * NE : (gg + 1) * NE]
                .unsqueeze(2)
                .broadcast_to([128, NE, O])
            )
            nc.vector.tensor_tensor(
                sc[:], yt_ps[:].rearrange("p (e o) -> p e o", e=NE), w_bc,
                op=OP.mult,
            )
            f1 = yout_p.tile([128, 4 * O], F32, tag="f1")
            nc.vector.tensor_add(f1[:], sc[:, 0:4, :], sc[:, 4:8, :])
            f2 = yout_p.tile([128, 2 * O], F32, tag="f2")
            nc.vector.tensor_add(
                f2[:], f1[:, 0 : 2 * O], f1[:, 2 * O : 4 * O]
            )
            nc.vector.tensor_add(
                yt_acc[:, gg * O : (gg + 1) * O],
                f2[:, 0:O], f2[:, O : 2 * O],
            )
        nc.gpsimd.dma_start(
            out[t * TT : (t + 1) * TT].rearrange("(gg p) o -> p gg o", p=128),
            yt_acc[:].rearrange("p (gg o) -> p gg o", gg=NG),
        )

    # ---------------- pipeline driver ----------------
    state = [dict(xts=[]) for _ in range(NT)]

    # tile 0 front runs alone (nothing to overlap with)
    for _ in front(0):
        pass
    # W1[1..7] DMAs enqueue between tile-0's and tile-1's x loads
    for e in range(1, NE):
        load_w1(e)

    for t in range(NT):
        exp_gen = experts(t)
        if t + 1 < NT:
            # interleave the next tile's front (24 chunk-steps) into this
            # tile's expert phase (8 expert-steps): 3 chunks per expert
            fr_gen = front(t + 1)
            for ei in range(NE):
                next(exp_gen, None)
                for _ in range(3):
                    next(fr_gen, None)
            for _ in fr_gen:
                pass
        else:
            for _ in exp_gen:
                pass
        # drain the generator (emits the trailing W2 matmuls)
        for _ in exp_gen:
            pass
        combine(t)


def _get_nc():
    if "nc" not in _CACHE:
        _CACHE["nc"] = _build_program()
    return _CACHE["nc"]


def _f16(a):
    return np.asarray(a, dtype=np.float32).astype(np.float16)


def kernel(x, Wg, bg, W1, b1, W2, b2, _trace=False, _tmpdir=None):
    nc = _get_nc()
    x = np.ascontiguousarray(np.asarray(x, dtype=np.float32))
    x_16 = _f16(x)

    Wg = np.asarray(Wg, dtype=np.float32)
    wg_hi = _f16(Wg)
    wg_lo = _f16(Wg - wg_hi.astype(np.float32))
    wgp = np.concatenate([wg_hi, wg_lo], axis=1)          # [D, 16]
    wgp = wgp.reshape(NCH, 128, 2 * NE).transpose(1, 0, 2)  # [128, NCH, 16]

    W2 = np.asarray(W2, dtype=np.float32)
    w2s = np.zeros((H, NE, NE * O), dtype=np.float16)
    for e in range(NE):
        w2s[:, e, O * e : O * (e + 1)] = _f16(W2[e])

    w1b = _f16(W1).reshape(NE, NCH, 128, H).transpose(0, 2, 1, 3)

    shared = {
        "Wgp": np.ascontiguousarray(wgp),
        "bg": np.ascontiguousarray(np.asarray(bg, dtype=np.float32)),
        "W1b": np.ascontiguousarray(w1b),
        "b1t": np.ascontiguousarray(np.asarray(b1, dtype=np.float32).T),
        "W2s": np.ascontiguousarray(w2s),
        "b2c": np.ascontiguousarray(
            np.asarray(b2, dtype=np.float32).reshape(NE * O)
        ),
    }
    in_maps = [
        {"x16": x_16[c * BC : (c + 1) * BC], **shared} for c in range(NCORES)
    ]
    res = bass_utils.run_bass_kernel_spmd(
        nc,
        in_maps,
        core_ids=list(range(NCORES)),
        trace=_trace,
        tmpdir=_tmpdir,
    )
    outp = np.concatenate([res.results[c]["out"] for c in range(NCORES)], axis=0)
    if _trace:
        kernel._last_results = res
    return outp


# revision 23
# speedup vs baseline: 1.2277x; 1.0212x over previous
"""MoE top-2 routing kernel for Trainium2 (8 NeuronCores, batch-sharded).

Problem (hardcoded shapes):
    x [8192, 3072] f32, Wg [3072, 8], bg [8], W1 [8, 3072, 128], b1 [8, 128],
    W2 [8, 128, 10], b2 [8, 10]  ->  out [8192, 10] f32
    g = x@Wg + bg; top-2 softmax over selected logits;
    y = sum_k w_k * (relu(x@W1[e_k] + b1[e_k]) @ W2[e_k] + b2[e_k])

Design (per core, 1024 tokens, dense over experts, PE-bound):
  - Host precasts x/W1/W2/Wg to fp16 (10-bit mantissa ~ tf32 quality)
    and pre-arranges every weight layout so all DMAs are contiguous.
  - x chunks are PE-transposed (fp16 moving, 1 cycle/row) and drained
    to SBUF fp16 by the scalar engine (values are already fp16 -
    the drain is exact).
  - Gating: stationary [Wg_hi | Wg_lo] (16 wide, host-built) removes
    the Wg rounding error; ONE matmul per chunk -> [16, 512] PSUM.
    The hi+lo fold happens token-major after the per-group transpose
    (DVE reads must start at partition 0).  fp16 x leaves a ~2^-11
    logit error: top-2 selection flips exactly 1 of 8192 tokens vs
    the fp32 reference (validated numerically); rel err 7.3e-3 vs
    the 2e-2 gate.
  - Expert h matmuls in fp16, fp32 PSUM.  relu(+b1) runs on the DVE
    as (h + b1) max 0.
  - W2 per expert is zero-padded into a [128, 80] stationary (expert e
    occupies columns 10e..10e+10): the 8 y matmuls accumulate disjoint
    10-partition stripes of ONE [80, 512] PSUM bank.  Per 128-token
    group: one PE transpose [80,128]->[128,80], then DVE: x w (free-dim
    broadcast over O), tree-sum over experts -> [128, 10].  b2 is a
    per-partition bias on the PSUM drain.
  - Software pipeline: tile t+1's front phase (x loads, transposes,
    gating) is interleaved into tile t's expert phase, so the PE never
    waits for data after the first tile fills.  W1 loads ride the sync
    queue between tile-0's x and tile-1's x, in expert order, so the
    expert loop streams behind the arrivals.
"""
import sys

for _p in ("/opt/trn_rl_repo",):
    if _p not in sys.path:
        sys.path.insert(0, _p)

import numpy as np
from contextlib import ExitStack

import concourse.bass as bass
import concourse.bacc as bacc
import concourse.tile as tile
import concourse.mybir as mybir
from concourse import bass_utils, masks

F32 = mybir.dt.float32
F16 = mybir.dt.float16
AF = mybir.ActivationFunctionType
OP = mybir.AluOpType

B, D, H, O, NE = 8192, 3072, 128, 10, 8
NCORES = 8
BC = B // NCORES          # tokens per core
TT = 512                  # token tile
NT = BC // TT             # token tiles per core
NCH = D // 128            # contraction chunks
NG = TT // 128            # 128-token groups per tile
SKEW = 3                  # gating trails the transposes by SKEW chunks

_CACHE = {}


def _build_program():
    nc = bacc.Bacc("TRN2", target_bir_lowering=False, debug=False,
                   num_devices=NCORES)

    x16 = nc.dram_tensor("x16", [BC, D], F16, kind="ExternalInput").ap()
    wgp = nc.dram_tensor("Wgp", [128, NCH, 128], F16, kind="ExternalInput").ap()
    bg = nc.dram_tensor("bg", [NE], F32, kind="ExternalInput").ap()
    w1 = nc.dram_tensor("W1b", [NE, 128, NCH, H], F16, kind="ExternalInput").ap()
    b1 = nc.dram_tensor("b1t", [H, NE], F32, kind="ExternalInput").ap()
    w2s = nc.dram_tensor("W2s", [H, NE, NE * O], F16, kind="ExternalInput").ap()
    b2c = nc.dram_tensor("b2c", [NE * O], F32, kind="ExternalInput").ap()
    out = nc.dram_tensor("out", [BC, O], F32, kind="ExternalOutput").ap()

    with tile.TileContext(nc) as tc:
        with ExitStack() as ctx:
            _kernel_body(ctx, tc, nc, x16, wgp, bg, w1, b1, w2s, b2c, out)
    nc.compile()
    return nc


def _kernel_body(ctx, tc, nc, x16, wgp, bg, w1, b1, w2s, b2c, out):
    singles = ctx.enter_context(tc.tile_pool(name="singles", bufs=1))
    xin_p = ctx.enter_context(tc.tile_pool(name="xin", bufs=4))
    xt_p = ctx.enter_context(tc.tile_pool(name="xt", bufs=2))
    gate_p = ctx.enter_context(tc.tile_pool(name="gate", bufs=2))
    hr_p = ctx.enter_context(tc.tile_pool(name="hr", bufs=2))
    yout_p = ctx.enter_context(tc.tile_pool(name="yout", bufs=2))

    ps_xtp = ctx.enter_context(tc.tile_pool(name="ps_xtp", bufs=2, space="PSUM"))
    ps_g = ctx.enter_context(tc.tile_pool(name="ps_g", bufs=1, space="PSUM"))
    ps_h = ctx.enter_context(tc.tile_pool(name="ps_h", bufs=2, space="PSUM"))
    ps_y = ctx.enter_context(tc.tile_pool(name="ps_y", bufs=1, space="PSUM"))
    ps_s = ctx.enter_context(tc.tile_pool(name="ps_s", bufs=2, space="PSUM"))

    # ---- constants (small, on the gpsimd DGE queue) ----
    ident = singles.tile([128, 128], F32)
    masks.make_identity(nc, ident[:])
    ident16 = singles.tile([128, 128], F16)
    nc.vector.tensor_copy(ident16[:], ident[:])

    # gating stationary zero-padded to 128 outputs: keeps the PE in the
    # same 128-wide column config as the h matmuls (a 32-wide tile
    # switch costs ~140ns per matmul)
    wg_pair = singles.tile([128, NCH, 128], F16)
    nc.gpsimd.dma_start(wg_pair[:], wgp)
    bg_row = singles.tile([1, NE], F32)
    nc.gpsimd.dma_start(bg_row[:], bg.rearrange("(one e) -> one e", one=1))
    bg_rep = singles.tile([128, NE], F32)
    nc.gpsimd.partition_broadcast(bg_rep[:], bg_row[:])
    b1t_sb = singles.tile([H, NE], F32)
    nc.gpsimd.dma_start(b1t_sb[:], b1)
    b2col = singles.tile([NE * O, 1], F32)
    nc.gpsimd.dma_start(b2col[:], b2c.rearrange("(p one) -> p one", one=1))
    w2st = singles.tile([H, NE, NE * O], F16)
    nc.gpsimd.dma_start(w2st[:], w2s)

    w1_sb = [
        singles.tile([128, NCH, H], F16, tag=f"w1_{e}", name=f"w1_{e}")
        for e in range(NE)
    ]

    def load_w1(e):
        nc.sync.dma_start(w1_sb[e][:], w1[e])

    # ---------------- pipeline stage generators ----------------
    # front(t): per chunk c yields after emitting the chunk's work.
    # Tile 0 loads token-major and PE-transposes (the XBAR can't beat
    # the PE into the pipeline at t=0); later tiles use the DMA XBAR
    # transpose on the sync queue - it runs during the previous tile's
    # expert phase, when the sync engine and queues are otherwise idle.
    # (The XBAR is a single shared unit - only ever one stream.)
    def front(t):
        tok0 = t * TT
        xts = state[t]["xts"]
        g_ps = ps_g.tile([128, TT], F32, tag="g", name=f"g{t}")
        state[t]["g_ps"] = g_ps

        def gating(cg):
            nc.tensor.matmul(
                g_ps[:], wg_pair[:, cg, :], xts[cg][:],
                start=(cg == 0), stop=(cg == NCH - 1),
            )

        if t > 0:
            for c in range(NCH):
                xt = xt_p.tile([128, TT], F16, tag=f"xt{c}", name=f"xt{c}")
                nc.sync.dma_start(
                    xt[:], x16[tok0 : tok0 + TT, c * 128 : (c + 1) * 128],
                    transpose=True,
                )
                xts.append(xt)
                gating(c)
                yield
            return

        for c in range(NCH):
            xin = xin_p.tile([128, NG, 128], F16, tag="xin")
            nc.sync.dma_start(
                xin[:],
                x16[tok0 : tok0 + TT, c * 128 : (c + 1) * 128].rearrange(
                    "(gg p) d -> p gg d", p=128
                ),
            )
            if c == 18:
                load_w1(0)  # W1[0] rides near the end of tile-0's x
            xtp = ps_xtp.tile([128, TT], F16, tag="xtp")
            for gg in range(NG):
                nc.tensor.matmul(
                    xtp[:, gg * 128 : (gg + 1) * 128],
                    xin[:, gg, :],
                    ident16[:],
                    is_transpose=True,
                    start=True,
                    stop=True,
                    skip_group_check=True,
                )
            xt = xt_p.tile([128, TT], F16, tag=f"xt{c}", name=f"xt{c}")
            nc.scalar.copy(xt[:], xtp[:])
            xts.append(xt)
            if c >= SKEW:
                gating(c - SKEW)
            yield
        for cg in range(NCH - SKEW, NCH):
            gating(cg)

    # experts(t): yields between experts; interleaves the epilogue and
    # the deferred W2 matmuls exactly as the PE should see them
    def experts(t):
        xts = state[t]["xts"]
        g_ps = state[t]["g_ps"]

        g_sb = gate_p.tile([2 * NE, TT], F32, tag="gsb")
        nc.vector.tensor_copy(g_sb[:], g_ps[0 : 2 * NE, :])

        y_ps = ps_y.tile([NE * O, TT], F32, tag="y", name=f"y{t}")
        wfull = gate_p.tile([128, NG * NE], F32, tag="wfull")
        state[t]["y_ps"] = y_ps
        state[t]["wfull"] = wfull
        hrs = {}

        def w2_mm(e):
            nc.tensor.matmul(
                y_ps[:], w2st[:, e, :], hrs.pop(e)[:],
                start=(e == 0), stop=(e == NE - 1), skip_group_check=True,
            )

        def epilogue():
            for gg in range(NG):
                gt_ps = ps_s.tile([128, 2 * NE], F32, tag="s")
                nc.tensor.transpose(
                    gt_ps[:], g_sb[:, gg * 128 : (gg + 1) * 128],
                    ident[0 : 2 * NE, 0 : 2 * NE],
                )
                gth = gate_p.tile([128, NE], F32, tag="gth")
                nc.vector.tensor_add(gth[:], gt_ps[:, 0:NE], bg_rep[:])
                gt = gate_p.tile([128, NE], F32, tag="gt")
                nc.vector.tensor_add(gt[:], gt_ps[:, NE : 2 * NE], gth[:])

                maxs = gate_p.tile([128, 8], F32, tag="maxs")
                nc.vector.max(maxs[:], gt[:])
                top1, top2 = maxs[:, 0:1], maxs[:, 1:2]

                sm = gate_p.tile([128, 4], F32, tag="sm")
                d21, e21, den, w2c = (sm[:, i : i + 1] for i in range(4))
                nc.vector.tensor_sub(d21, top2, top1)
                nc.scalar.activation(e21, d21, AF.Exp)
                nc.vector.tensor_scalar(den, e21, 1.0, None, OP.add)
                w1c = gate_p.tile([128, 1], F32, tag="w1c")
                nc.vector.reciprocal(w1c[:], den)
                nc.vector.tensor_mul(w2c, e21, w1c[:])

                m1 = gate_p.tile([128, NE], F32, tag="m1")
                m2 = gate_p.tile([128, NE], F32, tag="m2")
                nc.vector.tensor_scalar(m1[:], gt[:], top1, None, OP.is_equal)
                nc.vector.tensor_scalar(m2[:], gt[:], top2, None, OP.is_equal)
                nc.vector.tensor_scalar(m1[:], m1[:], w1c[:, 0:1], None, OP.mult)
                nc.vector.tensor_scalar(m2[:], m2[:], w2c, None, OP.mult)
                nc.vector.tensor_add(
                    wfull[:, gg * NE : (gg + 1) * NE], m1[:], m2[:]
                )

        for e in range(NE):
            h_ps = ps_h.tile([128, TT], F32, tag="h")
            for c in range(NCH):
                nc.tensor.matmul(
                    h_ps[:], w1_sb[e][:, c, :], xts[c][:],
                    start=(c == 0), stop=(c == NCH - 1),
                )
            if e == 0:
                epilogue()
            elif e >= 2:
                w2_mm(e - 2)
            hr = hr_p.tile([128, TT], F16, tag="hr", name=f"hr{t}_{e}")
            nc.vector.tensor_scalar(
                hr[:], h_ps[:], b1t_sb[:, e : e + 1], 0.0, OP.add, OP.max
            )
            hrs[e] = hr
            yield
        w2_mm(NE - 2)
        w2_mm(NE - 1)

    def combine(t):
        wfull = state[t]["wfull"]
        y_sb = yout_p.tile([NE * O, TT], F32, tag="ysb")
        nc.vector.tensor_scalar(
            y_sb[:], state[t]["y_ps"][:], b2col[:, 0:1], None, OP.add
        )
        yt_acc = yout_p.tile([128, NG * O], F32, tag="ytacc")
        for gg in range(NG):
            yt_ps = ps_s.tile([128, NE * O], F32, tag="s")
            nc.tensor.transpose(
                yt_ps[:], y_sb[:, gg * 128 : (gg + 1) * 128],
                ident[0 : NE * O, 0 : NE * O],
            )
            sc = yout_p.tile([128, NE, O], F32, tag="sc")
            w_bc = (
                wfull[:, gg * NE : (gg + 1) * NE]
                .unsqueeze(2)
                .broadcast_to([128, NE, O])
            )
            nc.vector.tensor_tensor(
                sc[:], yt_ps[:].rearrange("p (e o) -> p e o", e=NE), w_bc,
                op=OP.mult,
            )
            f1 = yout_p.tile([128, 4 * O], F32, tag="f1")
            nc.vector.tensor_add(f1[:], sc[:, 0:4, :], sc[:, 4:8, :])
            f2 = yout_p.tile([128, 2 * O], F32, tag="f2")
            nc.vector.tensor_add(
                f2[:], f1[:, 0 : 2 * O], f1[:, 2 * O : 4 * O]
            )
            nc.vector.tensor_add(
                yt_acc[:, gg * O : (gg + 1) * O],
                f2[:, 0:O], f2[:, O : 2 * O],
            )
        nc.gpsimd.dma_start(
            out[t * TT : (t + 1) * TT].rearrange("(gg p) o -> p gg o", p=128),
            yt_acc[:].rearrange("p (gg o) -> p gg o", gg=NG),
        )

    # ---------------- pipeline driver ----------------
    state = [dict(xts=[]) for _ in range(NT)]

    # tile 0 front runs alone (nothing to overlap with)
    for _ in front(0):
        pass
    # W1[1..7] DMAs enqueue between tile-0's and tile-1's x loads
    for e in range(1, NE):
        load_w1(e)

    for t in range(NT):
        exp_gen = experts(t)
        if t + 1 < NT:
            # interleave the next tile's front (24 chunk-steps) into this
            # tile's expert phase (8 expert-steps): 3 chunks per expert
            fr_gen = front(t + 1)
            for ei in range(NE):
                next(exp_gen, None)
                for _ in range(3):
                    next(fr_gen, None)
            for _ in fr_gen:
                pass
        else:
            for _ in exp_gen:
                pass
        # drain the generator (emits the trailing W2 matmuls)
        for _ in exp_gen:
            pass
        combine(t)


def _get_nc():
    if "nc" not in _CACHE:
        _CACHE["nc"] = _build_program()
    return _CACHE["nc"]


def _f16(a):
    return np.asarray(a, dtype=np.float32).astype(np.float16)


def kernel(x, Wg, bg, W1, b1, W2, b2, _trace=False, _tmpdir=None):
    nc = _get_nc()
    x = np.ascontiguousarray(np.asarray(x, dtype=np.float32))
    x_16 = _f16(x)

    Wg = np.asarray(Wg, dtype=np.float32)
    wg_hi = _f16(Wg)
    wg_lo = _f16(Wg - wg_hi.astype(np.float32))
    wgp = np.concatenate(
        [wg_hi, wg_lo, np.zeros((D, 128 - 2 * NE), np.float16)], axis=1
    )                                                     # [D, 128]
    wgp = wgp.reshape(NCH, 128, 128).transpose(1, 0, 2)   # [128, NCH, 128]

    W2 = np.asarray(W2, dtype=np.float32)
    w2s = np.zeros((H, NE, NE * O), dtype=np.float16)
    for e in range(NE):
        w2s[:, e, O * e : O * (e + 1)] = _f16(W2[e])

    w1b = _f16(W1).reshape(NE, NCH, 128, H).transpose(0, 2, 1, 3)

    shared = {
        "Wgp": np.ascontiguousarray(wgp),
        "bg": np.ascontiguousarray(np.asarray(bg, dtype=np.float32)),
        "W1b": np.ascontiguousarray(w1b),
        "b1t": np.ascontiguousarray(np.asarray(b1, dtype=np.float32).T),
        "W2s": np.ascontiguousarray(w2s),
        "b2c": np.ascontiguousarray(
            np.asarray(b2, dtype=np.float32).reshape(NE * O)
        ),
    }
    in_maps = [
        {"x16": x_16[c * BC : (c + 1) * BC], **shared} for c in range(NCORES)
    ]
    res = bass_utils.run_bass_kernel_spmd(
        nc,
        in_maps,
        core_ids=list(range(NCORES)),
        trace=_trace,
        tmpdir=_tmpdir,
    )
    outp = np.concatenate([res.results[c]["out"] for c in range(NCORES)], axis=0)
    if _trace:
        kernel._last_results = res
    return outp
